# revision 28
# baseline (speedup 1.0000x reference)
"""Trainium2 Bass kernel for the SNN leaky-integrate-and-fire problem.

Reference semantics (per batch row b, channels h=224, time t=224):
    x = roll(inp, 57, axis=time)
    T(b,t) = 3 + 2*tanh(dot(x[b,:,t], w))        (clip(1,5) is a no-op)
    mem(t) = beta*mem(t-1) + x(t) - T(t)*[mem(t-1) > T(t)]
    spk(t) = [mem(t) > T(t)]
    out[b, 0, h, t] = spk

Sharding: pure data parallelism over batch (1024 -> 8 cores x 128); the
128-row shard maps onto the 128 SBUF partitions, h rides the free dim and
the t recurrence runs as a sequence of [128, 224] ops.

Key design (vs the 400us 3-DVE-op/step baseline):

* State change of variables: track d(t) = mem(t) - T(t) instead of mem.
  Then spk(t) = [d(t) > 0] needs NO per-column bias, so spikes for a
  whole block are bulk sigmoid(2^100 * d) activations on the scalar
  engine instead of 224 per-column ones, and the per-(b,t) thresholds
  enter the recurrence only as the two per-partition scalar operands
  T_prev/T of a single fused step.

* The whole recurrence step is ONE custom-DVE instruction (LIF_STEP_ANT):
      v   = d + T_prev                  (reconstructs mem)
      d'  = ((v*beta + x) - T) - T*(v > T)
  with T_prev/T riding the two per-partition scalar slots and beta the
  immediate. DVE work drops from 3 instructions/step (~1.24us) to 1.
  Verified against the cached reference inputs: 2 / 51.4M spikes differ
  (rel err 1e-3, gate is 2e-2).

* Only ONE copy of the input is shipped (host pre-rolls and re-blocks to
  [b, nb, tb, ch] so each DVE column read is contiguous); the
  channel-major copy the baseline fed the PE is gone, halving HBM-in.

* The thresholds T(b,t) (dot + tanh + affine, 2% of the module FLOPs)
  are computed on the host and shipped as a [b, 225] f32 side input
  (115 KB per core): walrus rejects TensorScalarPtr on the GPSIMD/Pool
  engine, PE would need the channel-major copy back (DMA-bound), and
  the DVE is the critical engine, so every on-device placement loses
  60-90us. Shipping T directly (rather than the raw dots) also removes
  the DMA -> tanh -> affine chain from the kernel head; the first
  recurrence step is gated only by the first input chunk.
"""

import os
from contextlib import ExitStack

import numpy as np

import concourse.tile as tile
from concourse import bacc, bass_utils, mybir
from concourse.dve_ops import DveOp
from concourse.dve_spec import C0, C1, C2, Spec, Src0, Src1

F32 = mybir.dt.float32
U8 = mybir.dt.uint8
Act = mybir.ActivationFunctionType

CH = 224           # channels (h)
TT = 224           # time steps
ROLL = 57
BETA = 0.95
N_CORES = 8
BATCH = 1024
BPC = BATCH // N_CORES   # 128 = SBUF partitions
TB = 32            # time block
NB = TT // TB
SH = TB // 2       # spike half-block
QB = 4             # first-block DMA chunk (starts compute earlier)
BIG = float(2.0 ** 100)  # exact power-of-two spike sharpener


def _lif_ref(in0, in1, s0, s1, imm2):
    """Stage-exact numpy reference for LIF_STEP_ANT (CoreSim)."""
    f32 = np.float32
    v = (in0.astype(f32) + s0).astype(f32)
    h = (v > s1).astype(f32)
    out = ((v * f32(imm2)).astype(f32) + in1).astype(f32)
    out = (out - s1).astype(f32)
    return (out - (s1 * h).astype(f32)).astype(f32)


_V = Src0 + C0
LIF_STEP_ANT = DveOp(
    "LIF_STEP_ANT",
    Spec(body=((_V * C2 + Src1) - C1) - C1 * (_V > C1), reference=_lif_ref),
    subdim=False,
    uops_sha={"v3": "5c6b3c5ab6386ba1", "v4": "bf0ad38aa4655af5"},
)


def _register_lif_op():
    """Register LIF_STEP_ANT with the custom-DVE op registry (the public
    extension point is the OPS list; per-NEFF table gen + CoreSim resolve
    ops by name through it)."""
    from concourse import dve_ops

    if LIF_STEP_ANT.name in dve_ops._SUB_OPCODE_FOR_NAME:
        return
    row = max(dve_ops._SUB_OPCODE_FOR_NAME.values()) + 1
    assert row < 0x20, "custom-DVE row field overflow"
    dve_ops.OPS.append(LIF_STEP_ANT)
    dve_ops.CUSTOM_DVE_SPECS[LIF_STEP_ANT.name] = LIF_STEP_ANT.spec
    dve_ops._SUB_OPCODE_FOR_NAME[LIF_STEP_ANT.name] = row


_register_lif_op()


def lif_kernel(ctx, tc, out, inp, thrd, b=BPC, ch=CH, tb=TB, nb=NB):
    """Emit the LIF kernel body.

    inp:  [b, nb, tb, ch] f32  (host pre-rolled/blocked, batch-major)
    thrd: [b, nb*tb+1] f32     (host-computed thresholds; col 0 = T(-1) = 0)
    out:  [b, nb, tb, ch] u8 spikes
    """
    nc = tc.nc
    pers = ctx.enter_context(tc.tile_pool(name="pers", bufs=1))

    xb = [pers.tile([b, tb, ch], F32, tag=f"xb{i}", name=f"xb{i}")
          for i in range(3)]
    dblk = [pers.tile([b, tb, ch], F32, tag=f"d{i}", name=f"d{i}")
            for i in range(2)]
    spk = [pers.tile([b, tb, ch], U8, tag=f"spk{i}", name=f"spk{i}")
           for i in range(2)]
    thr = pers.tile([b, nb * tb + 1], F32, tag="thr")
    zcol = pers.tile([b, ch], F32, tag="zcol")

    # ---- prologue ----
    # The DMA engines drain queued descriptors FIFO, so everything is issued
    # on one queue in consumption order: the tiny threshold tensor first (it
    # gates the first step), then the first two blocks as small chunks
    # (block 0 fine-grained, block 1 coarser) — a monolithic block-1 load
    # would sit ahead of later block-0 chunks and stall the recurrence.
    nc.sync.dma_start(thr[:], thrd[:])
    # chunk sizes front-loaded small: the first columns gate the chain
    ofs = 0
    for w_ in [2, 2] + [QB] * ((tb - 4) // QB):
        nc.sync.dma_start(xb[0][:, ofs:ofs + w_, :], inp[:, 0, ofs:ofs + w_])
        ofs += w_
    for q in range(4):
        nc.sync.dma_start(xb[1][:, q * QB * 2:(q + 1) * QB * 2, :],
                          inp[:, 1, q * QB * 2:(q + 1) * QB * 2])
    nc.vector.memset(zcol[:], 0.0)

    for k in range(nb):
        if k + 2 < nb:
            nc.sync.dma_start(xb[(k + 2) % 3][:], inp[:, k + 2])
        dcur = dblk[k % 2]
        xcur = xb[k % 3]
        for tl in range(tb):
            t = k * tb + tl
            if k == 0 and tl == 0:
                prev = zcol[:]
            elif tl == 0:
                prev = dblk[(k - 1) % 2][:, tb - 1, :]
            else:
                prev = dcur[:, tl - 1, :]
            nc.vector._custom_dve(
                LIF_STEP_ANT,
                out=dcur[:, tl, :],
                in0=prev,
                in1=xcur[:, tl, :],
                s0=thr[:, t:t + 1],
                s1=thr[:, t + 1:t + 2],
                imm2=BETA,
            )
        # bulk spikes: sigmoid(2^100 * d) is exactly the > 0 indicator for
        # any representable nonzero d; the u8 store keeps the exact 0/1.
        # The last block goes in quarters to shorten the kernel tail.
        scur = spk[k % 2]
        # last block tapers to 2-column pieces so the final spike + store
        # land as soon as possible after the last recurrence step
        sizes = [4] * 7 + [2, 2] if k == nb - 1 else [SH, SH]
        ofs = 0
        for w_ in sizes:
            sl = slice(ofs, ofs + w_)
            nc.scalar.activation(scur[:, sl, :], dcur[:, sl, :],
                                 Act.Sigmoid, scale=BIG)
            nc.sync.dma_start(out[:, k, sl], scur[:, sl, :])
            ofs += w_


def build_kernel(b=BPC, ch=CH, tb=TB, nb=NB):
    nc = bacc.Bacc()
    inp = nc.dram_tensor("inp", [b, nb, tb, ch], F32, kind="ExternalInput")
    thrd = nc.dram_tensor("thrd", [b, nb * tb + 1], F32, kind="ExternalInput")
    out = nc.dram_tensor("out", [b, nb, tb, ch], U8, kind="ExternalOutput")

    with tile.TileContext(nc) as tc:
        with ExitStack() as ctx:
            lif_kernel(ctx, tc, out, inp, thrd, b=b, ch=ch, tb=tb, nb=nb)

    nc.compile()
    return nc


def host_pack(inp):
    """[B, ch, t] f32 -> rolled, time-blocked [B, nb, tb, ch]."""
    xr = np.roll(inp, ROLL, axis=2)
    xb = xr.reshape(inp.shape[0], CH, NB, TB).transpose(0, 2, 3, 1)
    return np.ascontiguousarray(xb)


def host_thr(inp, w):
    """[B, ch, t], [ch] -> thresholds [B, 1+t] with col 0 = T(-1) = 0."""
    f32 = np.float32
    xr = np.roll(inp, ROLL, axis=2)
    dots = np.tensordot(xr, w, axes=([1], [0])).astype(f32)
    T = np.clip(f32(3.0) + f32(2.0) * np.tanh(dots), 1.0, 5.0).astype(f32)
    out = np.zeros((T.shape[0], T.shape[1] + 1), f32)
    out[:, 1:] = T
    return out


def host_unpack(out_u8):
    """[B, nb, tb, ch] u8 spikes -> [B, 1, ch, t] f32."""
    o = out_u8.transpose(0, 3, 1, 2).reshape(out_u8.shape[0], CH, TT)
    return o.astype(np.float32)[:, None]


_NC_CACHE = {}


def _get_nc():
    key = "default"
    if key not in _NC_CACHE:
        _NC_CACHE[key] = build_kernel()
    return _NC_CACHE[key]


def kernel(inp: np.ndarray, w: np.ndarray) -> np.ndarray:
    inp = np.ascontiguousarray(inp, dtype=np.float32)
    w = np.ascontiguousarray(w, dtype=np.float32)
    assert inp.shape == (BATCH, CH, TT) and w.shape == (CH,)

    nc = _get_nc()
    packed = host_pack(inp)
    thr = host_thr(inp, w)
    in_maps = [
        {"inp": s, "thrd": t}
        for s, t in zip(np.split(packed, N_CORES, axis=0),
                        np.split(thr, N_CORES, axis=0))
    ]
    trace = bool(int(os.environ.get("LIF_TRACE", "0")))
    res = bass_utils.run_bass_kernel_spmd(
        nc, in_maps, core_ids=list(range(N_CORES)), trace=trace
    )
    kernel.last_results = res
    outs = [r["out"] for r in res.results]
    return host_unpack(np.concatenate(outs, axis=0))


# revision 30
# speedup vs baseline: 1.0016x; 1.0016x over previous
"""Trainium2 Bass kernel for the SNN leaky-integrate-and-fire problem.

Reference semantics (per batch row b, channels h=224, time t=224):
    x = roll(inp, 57, axis=time)
    T(b,t) = 3 + 2*tanh(dot(x[b,:,t], w))        (clip(1,5) is a no-op)
    mem(t) = beta*mem(t-1) + x(t) - T(t)*[mem(t-1) > T(t)]
    spk(t) = [mem(t) > T(t)]
    out[b, 0, h, t] = spk

Sharding: pure data parallelism over batch (1024 -> 8 cores x 128); the
128-row shard maps onto the 128 SBUF partitions, h rides the free dim and
the t recurrence runs as a sequence of [128, 224] ops.

Key design (vs the 400us 3-DVE-op/step baseline):

* State change of variables: track d(t) = mem(t) - T(t) instead of mem.
  Then spk(t) = [d(t) > 0] needs NO per-column bias, so spikes for a
  whole block are bulk sigmoid(2^100 * d) activations on the scalar
  engine instead of 224 per-column ones, and the per-(b,t) thresholds
  enter the recurrence only as the two per-partition scalar operands
  T_prev/T of a single fused step.

* The whole recurrence step is ONE custom-DVE instruction (LIF_STEP_ANT):
      v   = d + T_prev                  (reconstructs mem)
      d'  = ((v*beta + x) - T) - T*(v > T)
  with T_prev/T riding the two per-partition scalar slots and beta the
  immediate. DVE work drops from 3 instructions/step (~1.24us) to 1.
  Verified against the cached reference inputs: 2 / 51.4M spikes differ
  (rel err 1e-3, gate is 2e-2).

* Only ONE copy of the input is shipped (host pre-rolls and re-blocks to
  [b, nb, tb, ch] so each DVE column read is contiguous); the
  channel-major copy the baseline fed the PE is gone, halving HBM-in.

* The thresholds T(b,t) (dot + tanh + affine, 2% of the module FLOPs)
  are computed on the host and shipped as a [b, 225] f32 side input
  (115 KB per core): walrus rejects TensorScalarPtr on the GPSIMD/Pool
  engine, PE would need the channel-major copy back (DMA-bound), and
  the DVE is the critical engine, so every on-device placement loses
  60-90us. Shipping T directly (rather than the raw dots) also removes
  the DMA -> tanh -> affine chain from the kernel head; the first
  recurrence step is gated only by the first input chunk.
"""

import os
from contextlib import ExitStack

import numpy as np

import concourse.tile as tile
from concourse import bacc, bass_utils, mybir
from concourse.dve_ops import DveOp
from concourse.dve_spec import C0, C1, C2, Spec, Src0, Src1

F32 = mybir.dt.float32
U8 = mybir.dt.uint8
Act = mybir.ActivationFunctionType

CH = 224           # channels (h)
TT = 224           # time steps
ROLL = 57
BETA = 0.95
N_CORES = 8
BATCH = 1024
BPC = BATCH // N_CORES   # 128 = SBUF partitions
TB = 32            # time block
NB = TT // TB
SH = TB // 2       # spike half-block
QB = 4             # first-block DMA chunk (starts compute earlier)
BIG = float(2.0 ** 100)  # exact power-of-two spike sharpener


def _lif_ref(in0, in1, s0, s1, imm2):
    """Stage-exact numpy reference for LIF_STEP_ANT (CoreSim)."""
    f32 = np.float32
    v = (in0.astype(f32) + s0).astype(f32)
    h = (v > s1).astype(f32)
    out = ((v * f32(imm2)).astype(f32) + in1).astype(f32)
    out = (out - s1).astype(f32)
    return (out - (s1 * h).astype(f32)).astype(f32)


_V = Src0 + C0
LIF_STEP_ANT = DveOp(
    "LIF_STEP_ANT",
    Spec(body=((_V * C2 + Src1) - C1) - C1 * (_V > C1), reference=_lif_ref),
    subdim=False,
    uops_sha={"v3": "5c6b3c5ab6386ba1", "v4": "bf0ad38aa4655af5"},
)


def _register_lif_op():
    """Register LIF_STEP_ANT with the custom-DVE op registry (the public
    extension point is the OPS list; per-NEFF table gen + CoreSim resolve
    ops by name through it)."""
    from concourse import dve_ops

    if LIF_STEP_ANT.name in dve_ops._SUB_OPCODE_FOR_NAME:
        return
    row = max(dve_ops._SUB_OPCODE_FOR_NAME.values()) + 1
    assert row < 0x20, "custom-DVE row field overflow"
    dve_ops.OPS.append(LIF_STEP_ANT)
    dve_ops.CUSTOM_DVE_SPECS[LIF_STEP_ANT.name] = LIF_STEP_ANT.spec
    dve_ops._SUB_OPCODE_FOR_NAME[LIF_STEP_ANT.name] = row


_register_lif_op()


def lif_kernel(ctx, tc, out, inp, thrd, b=BPC, ch=CH, tb=TB, nb=NB):
    """Emit the LIF kernel body.

    inp:  [b, nb, tb, ch] f32  (host pre-rolled/blocked, batch-major)
    thrd: [b, nb*tb+1] f32     (host-computed thresholds; col 0 = T(-1) = 0)
    out:  [b, nb, tb, ch] u8 spikes
    """
    nc = tc.nc
    pers = ctx.enter_context(tc.tile_pool(name="pers", bufs=1))

    xb = [pers.tile([b, tb, ch], F32, tag=f"xb{i}", name=f"xb{i}")
          for i in range(3)]
    dblk = [pers.tile([b, tb, ch], F32, tag=f"d{i}", name=f"d{i}")
            for i in range(2)]
    spk = [pers.tile([b, tb, ch], U8, tag=f"spk{i}", name=f"spk{i}")
           for i in range(2)]
    thr = pers.tile([b, nb * tb + 1], F32, tag="thr")
    zcol = pers.tile([b, ch], F32, tag="zcol")

    # ---- prologue ----
    # The DMA engines drain queued descriptors FIFO, so everything is issued
    # on one queue in consumption order: the tiny threshold tensor first (it
    # gates the first step), then the first two blocks as small chunks
    # (block 0 fine-grained, block 1 coarser) — a monolithic block-1 load
    # would sit ahead of later block-0 chunks and stall the recurrence.
    nc.sync.dma_start(thr[:], thrd[:])
    nc.sync.dma_start(xb[0][:, 0:QB, :], inp[:, 0, 0:QB])
    for q in range(1, tb // QB):
        nc.sync.dma_start(xb[0][:, q * QB:(q + 1) * QB, :],
                          inp[:, 0, q * QB:(q + 1) * QB])
    for q in range(4):
        nc.sync.dma_start(xb[1][:, q * QB * 2:(q + 1) * QB * 2, :],
                          inp[:, 1, q * QB * 2:(q + 1) * QB * 2])
    nc.vector.memset(zcol[:], 0.0)

    for k in range(nb):
        if k + 2 < nb:
            nc.sync.dma_start(xb[(k + 2) % 3][:], inp[:, k + 2])
        dcur = dblk[k % 2]
        xcur = xb[k % 3]
        for tl in range(tb):
            t = k * tb + tl
            if k == 0 and tl == 0:
                prev = zcol[:]
            elif tl == 0:
                prev = dblk[(k - 1) % 2][:, tb - 1, :]
            else:
                prev = dcur[:, tl - 1, :]
            nc.vector._custom_dve(
                LIF_STEP_ANT,
                out=dcur[:, tl, :],
                in0=prev,
                in1=xcur[:, tl, :],
                s0=thr[:, t:t + 1],
                s1=thr[:, t + 1:t + 2],
                imm2=BETA,
            )
        # bulk spikes: sigmoid(2^100 * d) is exactly the > 0 indicator for
        # any representable nonzero d; the u8 store keeps the exact 0/1.
        # The last block goes in quarters to shorten the kernel tail.
        scur = spk[k % 2]
        pieces = 8 if k == nb - 1 else 2
        step = tb // pieces
        for h in range(pieces):
            sl = slice(h * step, (h + 1) * step)
            nc.scalar.activation(scur[:, sl, :], dcur[:, sl, :],
                                 Act.Sigmoid, scale=BIG)
            nc.sync.dma_start(out[:, k, sl], scur[:, sl, :])


def build_kernel(b=BPC, ch=CH, tb=TB, nb=NB):
    nc = bacc.Bacc()
    inp = nc.dram_tensor("inp", [b, nb, tb, ch], F32, kind="ExternalInput")
    thrd = nc.dram_tensor("thrd", [b, nb * tb + 1], F32, kind="ExternalInput")
    out = nc.dram_tensor("out", [b, nb, tb, ch], U8, kind="ExternalOutput")

    with tile.TileContext(nc) as tc:
        with ExitStack() as ctx:
            lif_kernel(ctx, tc, out, inp, thrd, b=b, ch=ch, tb=tb, nb=nb)

    nc.compile()
    return nc


def host_pack(inp):
    """[B, ch, t] f32 -> rolled, time-blocked [B, nb, tb, ch]."""
    xr = np.roll(inp, ROLL, axis=2)
    xb = xr.reshape(inp.shape[0], CH, NB, TB).transpose(0, 2, 3, 1)
    return np.ascontiguousarray(xb)


def host_thr(inp, w):
    """[B, ch, t], [ch] -> thresholds [B, 1+t] with col 0 = T(-1) = 0."""
    f32 = np.float32
    xr = np.roll(inp, ROLL, axis=2)
    dots = np.tensordot(xr, w, axes=([1], [0])).astype(f32)
    T = np.clip(f32(3.0) + f32(2.0) * np.tanh(dots), 1.0, 5.0).astype(f32)
    out = np.zeros((T.shape[0], T.shape[1] + 1), f32)
    out[:, 1:] = T
    return out


def host_unpack(out_u8):
    """[B, nb, tb, ch] u8 spikes -> [B, 1, ch, t] f32."""
    o = out_u8.transpose(0, 3, 1, 2).reshape(out_u8.shape[0], CH, TT)
    return o.astype(np.float32)[:, None]


_NC_CACHE = {}


def _get_nc():
    key = "default"
    if key not in _NC_CACHE:
        _NC_CACHE[key] = build_kernel()
    return _NC_CACHE[key]


def kernel(inp: np.ndarray, w: np.ndarray) -> np.ndarray:
    inp = np.ascontiguousarray(inp, dtype=np.float32)
    w = np.ascontiguousarray(w, dtype=np.float32)
    assert inp.shape == (BATCH, CH, TT) and w.shape == (CH,)

    nc = _get_nc()
    packed = host_pack(inp)
    thr = host_thr(inp, w)
    in_maps = [
        {"inp": s, "thrd": t}
        for s, t in zip(np.split(packed, N_CORES, axis=0),
                        np.split(thr, N_CORES, axis=0))
    ]
    trace = bool(int(os.environ.get("LIF_TRACE", "0")))
    res = bass_utils.run_bass_kernel_spmd(
        nc, in_maps, core_ids=list(range(N_CORES)), trace=trace
    )
    kernel.last_results = res
    outs = [r["out"] for r in res.results]
    return host_unpack(np.concatenate(outs, axis=0))
